# revision 33
# baseline (speedup 1.0000x reference)
"""Causal GQA self-attention (B=4, T=1024, D=2048, H=16, KVH=4, RoPE) on 8 TRN2 cores.

Sharding: 16 (batch, kv-group) units; core c handles batch c//2 and kv-groups
{2*(c%2), 2*(c%2)+1} (= 8 query heads). Wq/Wkv column-sharded, Wo row-sharded
(Megatron attention TP); each core returns a partial [T, D] output (bf16) and
the host sums the two partials per batch in f32.

v3 schedule (over v2):
- LDWEIGHTS dedup post-pass: consecutive matmuls sharing an identical
  stationary AP keep only the first LDWEIGHTS (saves the ~100-cycle weight
  swap per matmul).
- Loops restructured for stationary reuse: Q/K projections do both
  512-token halves per (head, kt) weight load; Wo is lh-outer with 4 PSUM
  column accumulators per token tile (one ot load covers 4 matmuls);
  softmax-denominator (lp) matmuls grouped into one ones-weight chain per
  (head, q-block); causal-mask matmuls grouped under one trineg load.
- Softmax tail restructured: raw AV output is copied PSUM->SBUF (bf16)
  immediately (frees the PSUM bank), and the 1/l divide happens in-place
  in SBUF later, so the gpsimd partition-broadcast never blocks the PE.
- ~38 dependency-free warm-up matmuls on scratch SBUF at t=0 keep the PE
  HAM clock warm through the initial DMA window (which otherwise runs the
  first ~12us of real matmuls at 1.2 GHz).
- wv/wk DMA'd in 4-6kt chunks so the first V/K matmuls gate on 256KB, not
  1MB; xt tiles round-robin over 3 rings in consumption order.
- Attention per head handles qb0+qb1 together right after that head's Q
  round; Wo runs as a final phase, double-buffered across token tiles.
"""

import sys

if "/opt/trn_rl_repo" not in sys.path:
    sys.path.insert(0, "/opt/trn_rl_repo")

from contextlib import ExitStack

import numpy as np

B, T, DIM = 4, 1024, 2048
H, KVH, HD = 16, 4, 128
G = H // KVH
P = 128
KO = DIM // P            # 16 contraction tiles
TT = T // P              # 8 token tiles
HPC = 8                  # heads per core
LG = 2                   # local kv groups per core
QBS = 512                # q block size
NQB = T // QBS           # 2
CBS = 512                # Wo col block size
NCB = DIM // CBS         # 4
SCALE = float(1.0 / np.sqrt(HD))
NCORES = 8
NDUMMY = 30              # PE warm-up matmuls (cover the DMA dead window)

_PROG_CACHE = {}


def _dedup_ldweights(nc, mybir):
    """Remove InstLdweights whose stationary AP + deps match the previous
    ldweights on the PE queue with only InstMatmults in between."""
    removed = 0
    for f in nc.m.functions:
        for b in f.blocks:
            insts = b.instructions
            last_key = None
            to_remove = []
            for i in insts:
                if isinstance(i, mybir.InstLdweights):
                    key = (
                        str(i.ins[0]),
                        str(i.perf_mode),
                        str(i.is_transpose),
                        str(i.tile_position),
                        tuple(sorted(i.sync_dependency_names())),
                        tuple(sorted(i.nosync_dependency_names())),
                    )
                    if key == last_key:
                        to_remove.append(i)
                    else:
                        last_key = key
                elif isinstance(i, mybir.InstMatmult):
                    pass
                elif getattr(i, "engine", None) == mybir.EngineType.PE:
                    last_key = None
            for i in to_remove:
                insts.remove(i)
                removed += 1
            b.instructions = insts
    return removed


def _build_program():
    import concourse.bacc as bacc
    import concourse.mybir as mybir
    import concourse.tile as tile

    f32 = mybir.dt.float32
    bf16 = mybir.dt.bfloat16
    EXP = mybir.ActivationFunctionType.Exp

    nc = bacc.Bacc("TRN2", debug=False)

    xt_d = nc.dram_tensor("xt", [P, KO, T], bf16, kind="ExternalInput").ap()
    wq_d = nc.dram_tensor("wq", [P, HPC, KO, HD], bf16, kind="ExternalInput").ap()
    wk_d = nc.dram_tensor("wk", [P, KO, LG * HD], bf16, kind="ExternalInput").ap()
    wv_d = nc.dram_tensor("wv", [P, KO, LG * HD], bf16, kind="ExternalInput").ap()
    wo_d = nc.dram_tensor("wo", [P, NCB, HPC, CBS], bf16, kind="ExternalInput").ap()
    cc_d = nc.dram_tensor("cc", [P, T], f32, kind="ExternalInput").ap()
    nss_d = nc.dram_tensor("nss", [P, T], f32, kind="ExternalInput").ap()
    tri_d = nc.dram_tensor("tri", [P, P], bf16, kind="ExternalInput").ap()
    trineg_d = nc.dram_tensor("trineg", [P, P], bf16, kind="ExternalInput").ap()
    idz_d = nc.dram_tensor("idz", [P, P], bf16, kind="ExternalInput").ap()
    swp_d = nc.dram_tensor("swp", [P, P], bf16, kind="ExternalInput").ap()
    y_d = nc.dram_tensor("y", [T, DIM], bf16, kind="ExternalOutput").ap()
    y_r = y_d.rearrange("(to p) c -> p to c", p=P)

    with tile.TileContext(nc) as tc, ExitStack() as ctx:
        const = ctx.enter_context(tc.tile_pool(name="const", bufs=1))
        xtp = ctx.enter_context(tc.tile_pool(name="xtp", bufs=1))
        big = ctx.enter_context(tc.tile_pool(name="big", bufs=1))
        ptp = ctx.enter_context(tc.tile_pool(name="ptp", bufs=8))
        tmp = ctx.enter_context(tc.tile_pool(name="tmp", bufs=2))
        rowp = ctx.enter_context(tc.tile_pool(name="rowp", bufs=4))
        rec128p = ctx.enter_context(tc.tile_pool(name="rec128p", bufs=2))
        ysbp = ctx.enter_context(tc.tile_pool(name="ysbp", bufs=4))

        # PSUM: 8 banks. ps_q: 1x[P,2,B] (2 banks), ps_sp: 2x[P,2,B]
        # (4 banks), ps_ol: 1x[P,2,B] (2 banks: op bank + lp bank).
        ps_q = ctx.enter_context(tc.tile_pool(name="ps_q", bufs=1, space="PSUM"))
        ps_sp = ctx.enter_context(tc.tile_pool(name="ps_sp", bufs=2, space="PSUM"))
        ps_ol = ctx.enter_context(tc.tile_pool(name="ps_ol", bufs=1, space="PSUM"))

        ccsb = const.tile([P, T], f32, tag="cc", name="cc")
        nsssb = const.tile([P, T], f32, tag="nss", name="nss")
        trisb = const.tile([P, P], bf16, tag="tri", name="tri")
        trinegsb = const.tile([P, P], bf16, tag="trineg", name="trineg")
        idzsb = const.tile([P, P], bf16, tag="idz", name="idz")
        swpsb = const.tile([P, P], bf16, tag="swp", name="swp")
        junk = const.tile([P, QBS], bf16, tag="junk", name="junk")
        ones_col = trisb[:, P - 1 : P]

        xtsb = xtp.tile([P, KO, T], bf16, tag="xt", name="xt")
        wqsb = big.tile([P, HPC, KO, HD], bf16, tag="wq", name="wqsb")
        wksb = big.tile([P, KO, LG * HD], bf16, tag="wk", name="wksb")
        wvsb = big.tile([P, KO, LG * HD], bf16, tag="wv", name="wvsb")
        wosb = big.tile([P, NCB, HPC, CBS], bf16, tag="wo", name="wosb")
        qtsb = [big.tile([P, T], bf16, tag=f"qt{h}", name=f"qt{h}") for h in range(HPC)]
        ktsb = big.tile([P, LG, T], bf16, tag="kt", name="kt")
        vsb = big.tile([P, TT, LG * HD], bf16, tag="v", name="v")
        otsb = qtsb  # OT_h overwrites QT_h per q-block after its last S read

        # ---- PE warm-up: dependency-free matmuls on scratch SBUF ----
        # They execute from ts~0 while input DMAs are in flight, flipping the
        # HAM clock gate to 8/8 before real work begins. Writes land in a
        # ps_q-pool tile that phase 1 reclaims afterward (in-order PE).
        warm = ps_q.tile([P, 2, QBS], f32, tag="q", name="warm")
        nc.gpsimd.memset(junk[:], 0.0)
        for i in range(NDUMMY):
            nc.tensor.matmul(
                warm[:, i % 2, :], junk[:, 0:P], junk[:], start=True, stop=True
            )

        # ---- DMA issue: consumption order over the 3 DMA-capable rings
        # (sync, scalar, gpsimd), ~100GB/s each. Per-ring cumulative loads
        # are tuned so every tensor lands just before first use. Rope swaps
        # ride scalar AFTER its ~2.4MB input tail; y outputs ride
        # sync+gpsimd at the very end.
        def xt_dma(eng, i):
            eng.dma_start(xtsb[:, i : i + 1, :], xt_d[:, i : i + 1, :])

        def wq_dma(eng, lh):
            eng.dma_start(wqsb[:, lh], wq_d[:, lh])

        # sync (~90GB/s): xt0..15(even-ish) cc wq2 wq4 wq6 wo[0:2]
        for i in (0, 2, 4, 6, 8, 10, 13, 15):
            xt_dma(nc.sync, i)
        nc.sync.dma_start(ccsb[:], cc_d)
        wq_dma(nc.sync, 2)
        wq_dma(nc.sync, 4)
        wq_dma(nc.sync, 6)
        nc.sync.dma_start(wosb[:, 0:2], wo_d[:, 0:2])
        # scalar (~85GB/s, rope swaps trail): wv0 wq0 wv1 wv2 nss wq1 tri..
        nc.scalar.dma_start(wvsb[:, 0:4], wv_d[:, 0:4])
        wq_dma(nc.scalar, 0)
        nc.scalar.dma_start(wvsb[:, 4:10], wv_d[:, 4:10])
        nc.scalar.dma_start(wvsb[:, 10:16], wv_d[:, 10:16])
        nc.scalar.dma_start(nsssb[:], nss_d)
        nc.scalar.dma_start(swpsb[:], swp_d)
        wq_dma(nc.scalar, 1)
        nc.scalar.dma_start(trisb[:], tri_d)
        nc.scalar.dma_start(trinegsb[:], trineg_d)
        nc.scalar.dma_start(idzsb[:], idz_d)
        # gpsimd (~150GB/s): wk + the other 10 xt tiles + wq3/5/7 + wo[2:4]
        nc.gpsimd.dma_start(wksb[:, 0:4], wk_d[:, 0:4])
        xt_dma(nc.gpsimd, 1)
        nc.gpsimd.dma_start(wksb[:, 4:10], wk_d[:, 4:10])
        xt_dma(nc.gpsimd, 3)
        xt_dma(nc.gpsimd, 5)
        nc.gpsimd.dma_start(wksb[:, 10:16], wk_d[:, 10:16])
        for i in (7, 9, 11, 12, 14):
            xt_dma(nc.gpsimd, i)
        wq_dma(nc.gpsimd, 3)
        wq_dma(nc.gpsimd, 5)
        wq_dma(nc.gpsimd, 7)
        nc.gpsimd.dma_start(wosb[:, 2:4], wo_d[:, 2:4])

        # ---- full-width rope (both 512-halves of a head at once) ----
        # The partition half-swap runs on the PE as a permutation matmul
        # (sw = swp.T @ usb) into an sp PSUM pair -- no SBUF-SBUF DMA, no
        # DMA-latency coupling into the vector queue.
        pend_rope = []  # (sw, t1, dst)

        def rope_evac(src_pair, split=False):
            """Copy the PSUM pair to SBUF (frees it). split=True uses
            scalar+vector in parallel so the pair frees in half the time."""
            usb = tmp.tile([P, T], bf16, tag="usb", name="usb")
            if split:
                nc.scalar.copy(usb[:, 0:QBS], src_pair[:, 0, :])
                nc.vector.tensor_scalar_mul(usb[:, QBS:T], src_pair[:, 1, :], 1.0)
            else:
                nc.scalar.copy(usb[:], src_pair[:, :, :])
            return usb

        def rope_swap(usb):
            """sw = half-swap(usb) via PE permutation; t1 = usb*cc."""
            sw = ps_sp.tile([P, 2, QBS], f32, tag="sp", name="sw")
            for j in range(2):
                nc.tensor.matmul(
                    sw[:, j, :],
                    swpsb[:],
                    usb[:, j * QBS : (j + 1) * QBS],
                    start=True,
                    stop=True,
                )
            t1 = tmp.tile([P, T], f32, tag="t1", name="t1")
            nc.vector.tensor_mul(t1[:], usb[:], ccsb[:])
            return sw, t1

        def rope_back(sw, t1, dst):
            t2 = tmp.tile([P, T], f32, tag="t2", name="t2")
            nc.vector.tensor_mul(t2[:], sw[:, :, :], nsssb[:])
            nc.vector.tensor_add(dst, t1[:], t2[:])

        # ---- attention for one head (both q-blocks) ----
        def attn_items(qb):
            items = [(kt, 0, False) for kt in range(4 * qb)]
            items += [(4 * qb + j, P * j, True) for j in range(4)]
            return items

        def attn_stage1(lh, qb, st):
            """Scores: full pairs exp right away; diag pairs share one
            grouped trineg mask chain, then exp."""
            lg = lh // 4
            base = qb * QBS
            items = attn_items(qb)
            npair = len(items) // 2
            pts = []
            diag_sps = []
            for pi in range(npair):
                pair = items[2 * pi : 2 * pi + 2]
                sp = ps_sp.tile([P, 2, QBS], f32, tag="sp", name="sp")
                for j, (kt, c0, diag) in enumerate(pair):
                    nc.tensor.matmul(
                        sp[:, j, c0:QBS],
                        ktsb[:, lg, kt * P : (kt + 1) * P],
                        qtsb[lh][:, base + c0 : base + QBS],
                        start=True,
                        stop=not diag,
                    )
                if not pair[0][2]:
                    pt = ptp.tile([P, 2, QBS], bf16, tag="pt", name="pt")
                    nc.scalar.activation(pt[:], sp[:], EXP, scale=SCALE)
                    pts.append(pt)
                else:
                    diag_sps.append((sp, pair))
            # grouped diag masks: one trineg load covers all diag items
            for sp, pair in diag_sps:
                for j, (kt, c0, diag) in enumerate(pair):
                    nc.tensor.matmul(
                        sp[:, j, c0 : c0 + P],
                        trinegsb[:],
                        idzsb[:],
                        start=False,
                        stop=True,
                    )
            for sp, pair in diag_sps:
                mn = min(c0 for (_, c0, _) in pair)
                pt = ptp.tile([P, 2, QBS], bf16, tag="pt", name="pt")
                nc.scalar.activation(
                    pt[:, :, mn:QBS], sp[:, :, mn:QBS], EXP, scale=SCALE
                )
                pts.append(pt)
            st["pairs"] = pts

        def attn_stage2(lh, qb, st):
            """Grouped lp chain (one ones-col load), then op chain."""
            lg = lh // 4
            items = attn_items(qb)
            nitems = len(items)
            ol = ps_ol.tile([P, 2, QBS], f32, tag="ol", name="ol")
            st["op"] = ol[:, 0, :]
            st["lp"] = ol[0:1, 1, :]
            for idx in range(nitems):
                kt, c0, diag = items[idx]
                pt = st["pairs"][idx // 2]
                nc.tensor.matmul(
                    st["lp"][:, c0:QBS],
                    ones_col,
                    pt[:, idx % 2, c0:QBS],
                    start=(idx == 0),
                    stop=(idx == nitems - 1),
                )
            for idx in range(nitems):
                kt, c0, diag = items[idx]
                pt = st["pairs"][idx // 2]
                nc.tensor.matmul(
                    st["op"][:, c0:QBS],
                    vsb[:, kt, lg * HD : (lg + 1) * HD],
                    pt[:, idx % 2, c0:QBS],
                    start=(idx == 0),
                    stop=(idx == nitems - 1),
                )

        def attn_tail(lh, qb, st):
            """Evacuate raw AV to SBUF (frees PSUM fast), then divide by l
            in-place in SBUF off the PE critical path."""
            base = qb * QBS
            dst = otsb[lh][:, base : base + QBS]
            nc.scalar.copy(dst, st["op"])
            scr = rowp.tile([1, QBS], f32, tag="row", name="scr")
            rec = rowp.tile([1, QBS], f32, tag="row", name="rec")
            nc.vector.reciprocal_approx_accurate(rec[:], st["lp"], scr[:])
            rec128 = rec128p.tile([P, QBS], f32, tag="rec128", name="rec128")
            nc.gpsimd.partition_broadcast(rec128[:], rec[:])
            nc.vector.tensor_mul(dst, dst, rec128[:])

        def attn_head(lh, mid_pe_hook=None):
            st0, st1 = {}, {}
            attn_stage1(lh, 0, st0)
            attn_stage1(lh, 1, st1)
            attn_stage2(lh, 0, st0)
            attn_tail(lh, 0, st0)
            if mid_pe_hook is not None:
                mid_pe_hook()
            attn_stage2(lh, 1, st1)
            attn_tail(lh, 1, st1)

        # ---- phase 1: V+K pass A (DMA-paced), then two double-buffered
        # V sub-passes. Pass A consumes each xt tile as it lands (4 V + 4 K
        # matmuls per kt ~= the DMA supply cadence). The tt4..7 sub-passes
        # alternate sp bufs so the evacuation copies (split across
        # scalar+vector) cost at most ~0.9us of PE wait at the boundary.
        kq = ps_q.tile([P, 2, QBS], f32, tag="q", name="kq")     # lg0 halves
        ko = ps_ol.tile([P, 2, QBS], f32, tag="ol", name="ko")   # lg1 halves
        kacc = [kq, ko]
        vA = [ps_sp.tile([P, 2, QBS], f32, tag="sp", name="vA") for _ in range(2)]
        vaccA = [vA[tt // 2][:, tt % 2, 0 : LG * HD] for tt in range(4)]
        for kt in range(KO):
            for tt in range(4):
                nc.tensor.matmul(
                    vaccA[tt],
                    xtsb[:, kt, tt * P : (tt + 1) * P],
                    wvsb[:, kt, :],
                    start=(kt == 0),
                    stop=(kt == KO - 1),
                )
            for lg in range(LG):
                for hf in range(2):
                    nc.tensor.matmul(
                        kacc[lg][:, hf, :],
                        wksb[:, kt, lg * HD : (lg + 1) * HD],
                        xtsb[:, kt, hf * QBS : (hf + 1) * QBS],
                        start=(kt == 0),
                        stop=(kt == KO - 1),
                    )
        nc.scalar.copy(vsb[:, 0:2, :], vA[0][:, :, 0 : LG * HD])
        nc.vector.tensor_scalar_mul(vsb[:, 2:4, :], vA[1][:, :, 0 : LG * HD], 1.0)
        kusb = [rope_evac(kacc[lg], split=True) for lg in range(LG)]
        for p in range(2):
            vp = ps_sp.tile([P, 2, QBS], f32, tag="sp", name=f"vB{p}")
            for kt in range(KO):
                for j in range(2):
                    tt = 4 + 2 * p + j
                    nc.tensor.matmul(
                        vp[:, j, 0 : LG * HD],
                        xtsb[:, kt, tt * P : (tt + 1) * P],
                        wvsb[:, kt, :],
                        start=(kt == 0),
                        stop=(kt == KO - 1),
                    )
            if p == 0:
                nc.scalar.copy(vsb[:, 4:6, :], vp[:, :, 0 : LG * HD])
            else:
                nc.vector.tensor_scalar_mul(
                    vsb[:, 6:8, :], vp[:, :, 0 : LG * HD], 1.0
                )
        for lg in range(LG):
            sw, t1 = rope_swap(kusb[lg])
            rope_back(sw, t1, ktsb[:, lg, :])

        # ---- phase 2: Q rounds (one head per round, paired halves) ----
        # Round r: proj(lh=r) 32 MMs (16 weight loads); rope_front(r);
        # rope_back(r-1); attn_head(r-1) fills PE while qp(r)'s usb copy
        # frees the pair for round r+1.
        for r in range(HPC):
            qp = ps_q.tile([P, 2, QBS], f32, tag="q", name="qp")
            usb = tmp.tile([P, T], bf16, tag="usb", name="usb")
            # interleave halves for kt<12 (shared weight loads); finish hf0
            # at kt 12..15 first so its evacuation overlaps the hf1 tail.
            for kt in range(12):
                for hf in range(2):
                    nc.tensor.matmul(
                        qp[:, hf, :],
                        wqsb[:, r, kt, :],
                        xtsb[:, kt, hf * QBS : (hf + 1) * QBS],
                        start=(kt == 0),
                        stop=False,
                    )
            for kt in range(12, KO):
                nc.tensor.matmul(
                    qp[:, 0, :],
                    wqsb[:, r, kt, :],
                    xtsb[:, kt, 0:QBS],
                    start=False,
                    stop=(kt == KO - 1),
                )
            nc.scalar.copy(usb[:, 0:QBS], qp[:, 0, :])
            for kt in range(12, KO):
                nc.tensor.matmul(
                    qp[:, 1, :],
                    wqsb[:, r, kt, :],
                    xtsb[:, kt, QBS:T],
                    start=False,
                    stop=(kt == KO - 1),
                )
            nc.vector.tensor_scalar_mul(usb[:, QBS:T], qp[:, 1, :], 1.0)
            if pend_rope:
                psw, pt1, pdst = pend_rope.pop()
                rope_back(psw, pt1, pdst)
            if r >= 1:
                attn_head(r - 1)
            sw, t1 = rope_swap(usb)
            pend_rope.append((sw, t1, qtsb[r][:, :]))
        sw, t1, pdst = pend_rope.pop()
        rope_back(sw, t1, pdst)

        # ---- phase 3: Wo, lh-outer with 4 column accumulators ----
        # Token tiles double-buffer: even tt uses the sp pool (2x2 banks),
        # odd tt uses ps_q + ps_ol (2+2 banks). One ot load -> 4 matmuls.
        # tt0's lh0..6 chains slot into attn_head(7)'s exp-ACT bubble
        # (they only need qb0 outputs of heads 0..6, long since final).
        dma_engines = [nc.sync, nc.gpsimd]
        wo_tt0 = {}

        def wo_tt0_head():
            ya = ps_q.tile([P, 2, QBS], f32, tag="q", name="wo_tt0a")
            yb = ps_sp.tile([P, 2, QBS], f32, tag="sp", name="wo_tt0b")
            wo_tt0["ya"], wo_tt0["yb"] = ya, yb
            yp = [ya[:, 0, :], ya[:, 1, :], yb[:, 0, :], yb[:, 1, :]]
            for lh in range(HPC - 1):
                for cb in range(NCB):
                    nc.tensor.matmul(
                        yp[cb],
                        otsb[lh][:, 0:P],
                        wosb[:, cb, lh, :],
                        start=(lh == 0),
                        stop=False,
                    )

        attn_head(HPC - 1, mid_pe_hook=wo_tt0_head)

        for tt in range(TT):
            if tt == 0:
                ya, yb = wo_tt0["ya"], wo_tt0["yb"]
            elif tt % 2 == 0:
                ya = ps_sp.tile([P, 2, QBS], f32, tag="sp", name="ya")
                yb = ps_sp.tile([P, 2, QBS], f32, tag="sp", name="yb")
            else:
                ya = ps_q.tile([P, 2, QBS], f32, tag="q", name="ya")
                yb = ps_ol.tile([P, 2, QBS], f32, tag="ol", name="yb")
            yp = [ya[:, 0, :], ya[:, 1, :], yb[:, 0, :], yb[:, 1, :]]
            for lh in range(0 if tt else HPC - 1, HPC):
                for cb in range(NCB):
                    nc.tensor.matmul(
                        yp[cb],
                        otsb[lh][:, tt * P : (tt + 1) * P],
                        wosb[:, cb, lh, :],
                        start=(lh == 0),
                        stop=(lh == HPC - 1),
                    )
            for cb in range(NCB):
                ysb = ysbp.tile([P, CBS], bf16, tag="ysb", name="ysb")
                if cb % 2 == 0:
                    nc.scalar.copy(ysb[:], yp[cb])
                else:
                    nc.vector.tensor_scalar_mul(ysb[:], yp[cb], 1.0)
                eng = dma_engines[(tt * NCB + cb) % 2]
                eng.dma_start(y_r[:, tt, cb * CBS : (cb + 1) * CBS], ysb[:])

    n_dedup = _dedup_ldweights(nc, mybir)
    nc.compile()
    return nc


def _get_program():
    if "p" not in _PROG_CACHE:
        _PROG_CACHE["p"] = _build_program()
    return _PROG_CACHE["p"]


def _prep_core(c, x, Wq, Wkv, Wo, cos, sin):
    import ml_dtypes

    mdt = ml_dtypes.bfloat16
    b = c // 2
    pair = c % 2
    groups = [2 * pair, 2 * pair + 1]
    heads = [g * G + i for g in groups for i in range(G)]

    xT = np.ascontiguousarray(x[b].T)                       # [DIM, T]
    xt_p = np.ascontiguousarray(xT.reshape(KO, P, T).transpose(1, 0, 2))

    wq_cols = np.stack([Wq[:, h * HD : (h + 1) * HD] for h in heads], axis=1)
    wq_p = np.ascontiguousarray(
        wq_cols.reshape(KO, P, HPC, HD).transpose(1, 2, 0, 3)
    )  # [P, lh, kt, c]

    kcols = np.concatenate([Wkv[:, g * HD : (g + 1) * HD] for g in groups], axis=1)
    wk_p = np.ascontiguousarray(kcols.reshape(KO, P, LG * HD).transpose(1, 0, 2))
    vcols = np.concatenate(
        [Wkv[:, KVH * HD + g * HD : KVH * HD + (g + 1) * HD] for g in groups], axis=1
    )
    wv_p = np.ascontiguousarray(vcols.reshape(KO, P, LG * HD).transpose(1, 0, 2))

    worows = np.stack([Wo[h * HD : (h + 1) * HD, :] for h in heads], axis=0)
    wo_p = np.ascontiguousarray(
        worows.reshape(HPC, P, NCB, CBS).transpose(1, 2, 0, 3)
    )  # [P, cb, lh, cc]

    cosT = np.ascontiguousarray(cos.T)                       # [64, T]
    sinT = np.ascontiguousarray(sin.T)
    cc_p = np.ascontiguousarray(np.concatenate([cosT, cosT], axis=0))   # [128, T]
    nss_p = np.ascontiguousarray(np.concatenate([-sinT, sinT], axis=0))
    tri_p = np.triu(np.ones((P, P), dtype=np.float32))
    trineg_p = -32768.0 * np.triu(np.ones((P, P), dtype=np.float32), k=1)
    idz_p = np.eye(P, dtype=np.float32)
    swp_p = np.roll(np.eye(P, dtype=np.float32), 64, axis=0)

    return {
        "xt": xt_p.astype(mdt),
        "wq": wq_p.astype(mdt),
        "wk": wk_p.astype(mdt),
        "wv": wv_p.astype(mdt),
        "wo": wo_p.astype(mdt),
        "cc": cc_p.astype(np.float32, copy=False),
        "nss": nss_p.astype(np.float32, copy=False),
        "tri": tri_p.astype(mdt),
        "trineg": trineg_p.astype(mdt),
        "idz": idz_p.astype(mdt),
        "swp": swp_p.astype(mdt),
    }


def _run(inputs, trace=False, trace_kwargs=None):
    from concourse import bass_utils

    x = np.asarray(inputs["x"], dtype=np.float32)
    Wq = np.asarray(inputs["Wq"], dtype=np.float32)
    Wkv = np.asarray(inputs["Wkv"], dtype=np.float32)
    Wo = np.asarray(inputs["Wo"], dtype=np.float32)
    cos = np.asarray(inputs["cos"], dtype=np.float32)
    sin = np.asarray(inputs["sin"], dtype=np.float32)

    nc = _get_program()
    in_maps = [_prep_core(c, x, Wq, Wkv, Wo, cos, sin) for c in range(NCORES)]
    kwargs = {}
    if trace:
        kwargs["trace"] = True
        if trace_kwargs:
            kwargs.update(trace_kwargs)
    res = bass_utils.run_bass_kernel_spmd(
        nc, in_maps, core_ids=list(range(NCORES)), **kwargs
    )
    outs = [np.asarray(r["y"]).astype(np.float32) for r in res.results]
    y = np.stack([outs[2 * b] + outs[2 * b + 1] for b in range(B)], axis=0)
    return y, res


def kernel(**inputs):
    y, _ = _run(inputs, trace=False)
    return y


# revision 35
# speedup vs baseline: 1.0081x; 1.0081x over previous
"""Causal GQA self-attention (B=4, T=1024, D=2048, H=16, KVH=4, RoPE) on 8 TRN2 cores.

Sharding: 16 (batch, kv-group) units; core c handles batch c//2 and kv-groups
{2*(c%2), 2*(c%2)+1} (= 8 query heads). Wq/Wkv column-sharded, Wo row-sharded
(Megatron attention TP); each core returns a partial [T, D] output (bf16) and
the host sums the two partials per batch in f32.

v3 schedule (over v2):
- LDWEIGHTS dedup post-pass: consecutive matmuls sharing an identical
  stationary AP keep only the first LDWEIGHTS (saves the ~100-cycle weight
  swap per matmul).
- Loops restructured for stationary reuse: Q/K projections do both
  512-token halves per (head, kt) weight load; Wo is lh-outer with 4 PSUM
  column accumulators per token tile (one ot load covers 4 matmuls);
  softmax-denominator (lp) matmuls grouped into one ones-weight chain per
  (head, q-block); causal-mask matmuls grouped under one trineg load.
- Softmax tail restructured: raw AV output is copied PSUM->SBUF (bf16)
  immediately (frees the PSUM bank), and the 1/l divide happens in-place
  in SBUF later, so the gpsimd partition-broadcast never blocks the PE.
- ~38 dependency-free warm-up matmuls on scratch SBUF at t=0 keep the PE
  HAM clock warm through the initial DMA window (which otherwise runs the
  first ~12us of real matmuls at 1.2 GHz).
- wv/wk DMA'd in 4-6kt chunks so the first V/K matmuls gate on 256KB, not
  1MB; xt tiles round-robin over 3 rings in consumption order.
- Attention per head handles qb0+qb1 together right after that head's Q
  round; Wo runs as a final phase, double-buffered across token tiles.
"""

import sys

if "/opt/trn_rl_repo" not in sys.path:
    sys.path.insert(0, "/opt/trn_rl_repo")

from contextlib import ExitStack

import numpy as np

B, T, DIM = 4, 1024, 2048
H, KVH, HD = 16, 4, 128
G = H // KVH
P = 128
KO = DIM // P            # 16 contraction tiles
TT = T // P              # 8 token tiles
HPC = 8                  # heads per core
LG = 2                   # local kv groups per core
QBS = 512                # q block size
NQB = T // QBS           # 2
CBS = 512                # Wo col block size
NCB = DIM // CBS         # 4
SCALE = float(1.0 / np.sqrt(HD))
NCORES = 8
NDUMMY = 28              # PE warm-up matmuls (cover the DMA dead window)

_PROG_CACHE = {}


def _dedup_ldweights(nc, mybir):
    """Remove InstLdweights whose stationary AP + deps match the previous
    ldweights on the PE queue with only InstMatmults in between."""
    removed = 0
    for f in nc.m.functions:
        for b in f.blocks:
            insts = b.instructions
            last_key = None
            to_remove = []
            for i in insts:
                if isinstance(i, mybir.InstLdweights):
                    key = (
                        str(i.ins[0]),
                        str(i.perf_mode),
                        str(i.is_transpose),
                        str(i.tile_position),
                        tuple(sorted(i.sync_dependency_names())),
                        tuple(sorted(i.nosync_dependency_names())),
                    )
                    if key == last_key:
                        to_remove.append(i)
                    else:
                        last_key = key
                elif isinstance(i, mybir.InstMatmult):
                    pass
                elif getattr(i, "engine", None) == mybir.EngineType.PE:
                    last_key = None
            for i in to_remove:
                insts.remove(i)
                removed += 1
            b.instructions = insts
    return removed


def _build_program():
    import concourse.bacc as bacc
    import concourse.mybir as mybir
    import concourse.tile as tile

    f32 = mybir.dt.float32
    bf16 = mybir.dt.bfloat16
    EXP = mybir.ActivationFunctionType.Exp

    nc = bacc.Bacc("TRN2", debug=False)

    xt_d = nc.dram_tensor("xt", [P, KO, T], bf16, kind="ExternalInput").ap()
    wq_d = nc.dram_tensor("wq", [P, HPC, KO, HD], bf16, kind="ExternalInput").ap()
    wk_d = nc.dram_tensor("wk", [P, KO, LG * HD], bf16, kind="ExternalInput").ap()
    wv_d = nc.dram_tensor("wv", [P, KO, LG * HD], bf16, kind="ExternalInput").ap()
    wo_d = nc.dram_tensor("wo", [P, NCB, HPC, CBS], bf16, kind="ExternalInput").ap()
    cc_d = nc.dram_tensor("cc", [P, T], f32, kind="ExternalInput").ap()
    nss_d = nc.dram_tensor("nss", [P, T], f32, kind="ExternalInput").ap()
    tri_d = nc.dram_tensor("tri", [P, P], bf16, kind="ExternalInput").ap()
    trineg_d = nc.dram_tensor("trineg", [P, P], bf16, kind="ExternalInput").ap()
    idz_d = nc.dram_tensor("idz", [P, P], bf16, kind="ExternalInput").ap()
    swp_d = nc.dram_tensor("swp", [P, P], bf16, kind="ExternalInput").ap()
    y_d = nc.dram_tensor("y", [T, DIM], bf16, kind="ExternalOutput").ap()
    y_r = y_d.rearrange("(to p) c -> p to c", p=P)

    with tile.TileContext(nc) as tc, ExitStack() as ctx:
        const = ctx.enter_context(tc.tile_pool(name="const", bufs=1))
        xtp = ctx.enter_context(tc.tile_pool(name="xtp", bufs=1))
        big = ctx.enter_context(tc.tile_pool(name="big", bufs=1))
        ptp = ctx.enter_context(tc.tile_pool(name="ptp", bufs=8))
        tmp = ctx.enter_context(tc.tile_pool(name="tmp", bufs=2))
        rowp = ctx.enter_context(tc.tile_pool(name="rowp", bufs=4))
        rec128p = ctx.enter_context(tc.tile_pool(name="rec128p", bufs=2))
        ysbp = ctx.enter_context(tc.tile_pool(name="ysbp", bufs=4))

        # PSUM: 8 banks. ps_q: 1x[P,2,B] (2 banks), ps_sp: 2x[P,2,B]
        # (4 banks), ps_ol: 1x[P,2,B] (2 banks: op bank + lp bank).
        ps_q = ctx.enter_context(tc.tile_pool(name="ps_q", bufs=1, space="PSUM"))
        ps_sp = ctx.enter_context(tc.tile_pool(name="ps_sp", bufs=2, space="PSUM"))
        ps_ol = ctx.enter_context(tc.tile_pool(name="ps_ol", bufs=1, space="PSUM"))

        ccsb = const.tile([P, T], f32, tag="cc", name="cc")
        nsssb = const.tile([P, T], f32, tag="nss", name="nss")
        trisb = const.tile([P, P], bf16, tag="tri", name="tri")
        trinegsb = const.tile([P, P], bf16, tag="trineg", name="trineg")
        idzsb = const.tile([P, P], bf16, tag="idz", name="idz")
        swpsb = const.tile([P, P], bf16, tag="swp", name="swp")
        junk = const.tile([P, QBS], bf16, tag="junk", name="junk")
        ones_col = trisb[:, P - 1 : P]

        xtsb = xtp.tile([P, KO, T], bf16, tag="xt", name="xt")
        wqsb = big.tile([P, HPC, KO, HD], bf16, tag="wq", name="wqsb")
        wksb = big.tile([P, KO, LG * HD], bf16, tag="wk", name="wksb")
        wvsb = big.tile([P, KO, LG * HD], bf16, tag="wv", name="wvsb")
        wosb = big.tile([P, NCB, HPC, CBS], bf16, tag="wo", name="wosb")
        qtsb = [big.tile([P, T], bf16, tag=f"qt{h}", name=f"qt{h}") for h in range(HPC)]
        ktsb = big.tile([P, LG, T], bf16, tag="kt", name="kt")
        vsb = big.tile([P, TT, LG * HD], bf16, tag="v", name="v")
        otsb = qtsb  # OT_h overwrites QT_h per q-block after its last S read

        # ---- PE warm-up: dependency-free matmuls on scratch SBUF ----
        # They execute from ts~0 while input DMAs are in flight, flipping the
        # HAM clock gate to 8/8 before real work begins. Writes land in a
        # ps_q-pool tile that phase 1 reclaims afterward (in-order PE).
        warm = ps_q.tile([P, 2, QBS], f32, tag="q", name="warm")
        nc.gpsimd.memset(junk[:], 0.0)
        for i in range(NDUMMY):
            nc.tensor.matmul(
                warm[:, i % 2, :], junk[:, 0:P], junk[:], start=True, stop=True
            )

        # ---- DMA issue: consumption order over the 3 DMA-capable rings
        # (sync, scalar, gpsimd), ~100GB/s each. Per-ring cumulative loads
        # are tuned so every tensor lands just before first use. Rope swaps
        # ride scalar AFTER its ~2.4MB input tail; y outputs ride
        # sync+gpsimd at the very end.
        def xt_dma(eng, i):
            eng.dma_start(xtsb[:, i : i + 1, :], xt_d[:, i : i + 1, :])

        def wq_dma(eng, lh):
            eng.dma_start(wqsb[:, lh], wq_d[:, lh])

        # sync (~90GB/s): xt0..15(even-ish) cc wq2 wq4 wq6 wo[0:2]
        for i in (0, 2, 4, 6, 8, 10, 13, 15):
            xt_dma(nc.sync, i)
        nc.sync.dma_start(ccsb[:], cc_d)
        wq_dma(nc.sync, 2)
        wq_dma(nc.sync, 4)
        wq_dma(nc.sync, 6)
        nc.sync.dma_start(wosb[:, 0:2], wo_d[:, 0:2])
        # scalar (~85GB/s, rope swaps trail): wv0 wq0 wv1 wv2 nss wq1 tri..
        nc.scalar.dma_start(wvsb[:, 0:4], wv_d[:, 0:4])
        wq_dma(nc.scalar, 0)
        nc.scalar.dma_start(wvsb[:, 4:10], wv_d[:, 4:10])
        nc.scalar.dma_start(wvsb[:, 10:16], wv_d[:, 10:16])
        nc.scalar.dma_start(nsssb[:], nss_d)
        nc.scalar.dma_start(swpsb[:], swp_d)
        wq_dma(nc.scalar, 1)
        nc.scalar.dma_start(trisb[:], tri_d)
        nc.scalar.dma_start(trinegsb[:], trineg_d)
        nc.scalar.dma_start(idzsb[:], idz_d)
        # gpsimd (~150GB/s): wk + the other 10 xt tiles + wq3/5/7 + wo[2:4]
        nc.gpsimd.dma_start(wksb[:, 0:4], wk_d[:, 0:4])
        xt_dma(nc.gpsimd, 1)
        nc.gpsimd.dma_start(wksb[:, 4:10], wk_d[:, 4:10])
        xt_dma(nc.gpsimd, 3)
        xt_dma(nc.gpsimd, 5)
        nc.gpsimd.dma_start(wksb[:, 10:16], wk_d[:, 10:16])
        for i in (7, 9, 11, 12, 14):
            xt_dma(nc.gpsimd, i)
        wq_dma(nc.gpsimd, 3)
        wq_dma(nc.gpsimd, 5)
        wq_dma(nc.gpsimd, 7)
        nc.gpsimd.dma_start(wosb[:, 2:4], wo_d[:, 2:4])

        # ---- full-width rope (both 512-halves of a head at once) ----
        # The partition half-swap runs on the PE as a permutation matmul
        # (sw = swp.T @ usb) into an sp PSUM pair -- no SBUF-SBUF DMA, no
        # DMA-latency coupling into the vector queue.
        pend_rope = []  # (sw, t1, dst)

        def rope_evac(src_pair, split=False):
            """Copy the PSUM pair to SBUF (frees it). split=True uses
            scalar+vector in parallel so the pair frees in half the time."""
            usb = tmp.tile([P, T], bf16, tag="usb", name="usb")
            if split:
                nc.scalar.copy(usb[:, 0:QBS], src_pair[:, 0, :])
                nc.vector.tensor_scalar_mul(usb[:, QBS:T], src_pair[:, 1, :], 1.0)
            else:
                nc.scalar.copy(usb[:], src_pair[:, :, :])
            return usb

        def rope_swap(usb):
            """sw = half-swap(usb) via PE permutation; t1 = usb*cc."""
            sw = ps_sp.tile([P, 2, QBS], f32, tag="sp", name="sw")
            for j in range(2):
                nc.tensor.matmul(
                    sw[:, j, :],
                    swpsb[:],
                    usb[:, j * QBS : (j + 1) * QBS],
                    start=True,
                    stop=True,
                )
            t1 = tmp.tile([P, T], f32, tag="t1", name="t1")
            nc.vector.tensor_mul(t1[:], usb[:], ccsb[:])
            return sw, t1

        def rope_back(sw, t1, dst):
            t2 = tmp.tile([P, T], f32, tag="t2", name="t2")
            nc.vector.tensor_mul(t2[:], sw[:, :, :], nsssb[:])
            nc.vector.tensor_add(dst, t1[:], t2[:])

        # ---- attention for one head (both q-blocks) ----
        def attn_items(qb):
            items = [(kt, 0, False) for kt in range(4 * qb)]
            items += [(4 * qb + j, P * j, True) for j in range(4)]
            return items

        def attn_stage1(lh, qb, st):
            """Scores: full pairs exp right away; diag pairs share one
            grouped trineg mask chain, then exp."""
            lg = lh // 4
            base = qb * QBS
            items = attn_items(qb)
            npair = len(items) // 2
            pts = []
            diag_sps = []
            for pi in range(npair):
                pair = items[2 * pi : 2 * pi + 2]
                sp = ps_sp.tile([P, 2, QBS], f32, tag="sp", name="sp")
                for j, (kt, c0, diag) in enumerate(pair):
                    nc.tensor.matmul(
                        sp[:, j, c0:QBS],
                        ktsb[:, lg, kt * P : (kt + 1) * P],
                        qtsb[lh][:, base + c0 : base + QBS],
                        start=True,
                        stop=not diag,
                    )
                if not pair[0][2]:
                    pt = ptp.tile([P, 2, QBS], bf16, tag="pt", name="pt")
                    nc.scalar.activation(pt[:], sp[:], EXP, scale=SCALE)
                    pts.append(pt)
                else:
                    diag_sps.append((sp, pair))
            # grouped diag masks: one trineg load covers all diag items
            for sp, pair in diag_sps:
                for j, (kt, c0, diag) in enumerate(pair):
                    nc.tensor.matmul(
                        sp[:, j, c0 : c0 + P],
                        trinegsb[:],
                        idzsb[:],
                        start=False,
                        stop=True,
                    )
            for sp, pair in diag_sps:
                mn = min(c0 for (_, c0, _) in pair)
                pt = ptp.tile([P, 2, QBS], bf16, tag="pt", name="pt")
                nc.scalar.activation(
                    pt[:, :, mn:QBS], sp[:, :, mn:QBS], EXP, scale=SCALE
                )
                pts.append(pt)
            st["pairs"] = pts

        def attn_stage2(lh, qb, st):
            """Grouped lp chain (one ones-col load), then op chain."""
            lg = lh // 4
            items = attn_items(qb)
            nitems = len(items)
            ol = ps_ol.tile([P, 2, QBS], f32, tag="ol", name="ol")
            st["op"] = ol[:, 0, :]
            st["lp"] = ol[0:1, 1, :]
            for idx in range(nitems):
                kt, c0, diag = items[idx]
                pt = st["pairs"][idx // 2]
                nc.tensor.matmul(
                    st["lp"][:, c0:QBS],
                    ones_col,
                    pt[:, idx % 2, c0:QBS],
                    start=(idx == 0),
                    stop=(idx == nitems - 1),
                )
            for idx in range(nitems):
                kt, c0, diag = items[idx]
                pt = st["pairs"][idx // 2]
                nc.tensor.matmul(
                    st["op"][:, c0:QBS],
                    vsb[:, kt, lg * HD : (lg + 1) * HD],
                    pt[:, idx % 2, c0:QBS],
                    start=(idx == 0),
                    stop=(idx == nitems - 1),
                )

        def attn_tail(lh, qb, st):
            """Evacuate raw AV to SBUF (frees PSUM fast), then divide by l
            in-place in SBUF off the PE critical path."""
            base = qb * QBS
            dst = otsb[lh][:, base : base + QBS]
            nc.scalar.copy(dst, st["op"])
            scr = rowp.tile([1, QBS], f32, tag="row", name="scr")
            rec = rowp.tile([1, QBS], f32, tag="row", name="rec")
            nc.vector.reciprocal_approx_accurate(rec[:], st["lp"], scr[:])
            rec128 = rec128p.tile([P, QBS], f32, tag="rec128", name="rec128")
            nc.gpsimd.partition_broadcast(rec128[:], rec[:])
            nc.vector.tensor_mul(dst, dst, rec128[:])

        def attn_head(lh, mid_pe_hook=None):
            st0, st1 = {}, {}
            attn_stage1(lh, 0, st0)
            attn_stage1(lh, 1, st1)
            attn_stage2(lh, 0, st0)
            attn_tail(lh, 0, st0)
            if mid_pe_hook is not None:
                mid_pe_hook()
            attn_stage2(lh, 1, st1)
            attn_tail(lh, 1, st1)

        # ---- phase 1: V+K pass A (DMA-paced), then two double-buffered
        # V sub-passes. Pass A consumes each xt tile as it lands (4 V + 4 K
        # matmuls per kt ~= the DMA supply cadence). The tt4..7 sub-passes
        # alternate sp bufs so the evacuation copies (split across
        # scalar+vector) cost at most ~0.9us of PE wait at the boundary.
        kq = ps_q.tile([P, 2, QBS], f32, tag="q", name="kq")     # lg0 halves
        ko = ps_ol.tile([P, 2, QBS], f32, tag="ol", name="ko")   # lg1 halves
        kacc = [kq, ko]
        vA = [ps_sp.tile([P, 2, QBS], f32, tag="sp", name="vA") for _ in range(2)]
        vaccA = [vA[tt // 2][:, tt % 2, 0 : LG * HD] for tt in range(4)]
        for kt in range(KO):
            for tt in range(4):
                nc.tensor.matmul(
                    vaccA[tt],
                    xtsb[:, kt, tt * P : (tt + 1) * P],
                    wvsb[:, kt, :],
                    start=(kt == 0),
                    stop=(kt == KO - 1),
                )
            for lg in range(LG):
                for hf in range(2):
                    nc.tensor.matmul(
                        kacc[lg][:, hf, :],
                        wksb[:, kt, lg * HD : (lg + 1) * HD],
                        xtsb[:, kt, hf * QBS : (hf + 1) * QBS],
                        start=(kt == 0),
                        stop=(kt == KO - 1),
                    )
        nc.scalar.copy(vsb[:, 0:2, :], vA[0][:, :, 0 : LG * HD])
        nc.vector.tensor_scalar_mul(vsb[:, 2:4, :], vA[1][:, :, 0 : LG * HD], 1.0)
        kusb = [rope_evac(kacc[lg], split=True) for lg in range(LG)]
        for p in range(2):
            vp = ps_sp.tile([P, 2, QBS], f32, tag="sp", name=f"vB{p}")
            for kt in range(KO):
                for j in range(2):
                    tt = 4 + 2 * p + j
                    nc.tensor.matmul(
                        vp[:, j, 0 : LG * HD],
                        xtsb[:, kt, tt * P : (tt + 1) * P],
                        wvsb[:, kt, :],
                        start=(kt == 0),
                        stop=(kt == KO - 1),
                    )
            if p == 0:
                nc.scalar.copy(vsb[:, 4:6, :], vp[:, :, 0 : LG * HD])
            else:
                nc.vector.tensor_scalar_mul(
                    vsb[:, 6:8, :], vp[:, :, 0 : LG * HD], 1.0
                )
        for lg in range(LG):
            sw, t1 = rope_swap(kusb[lg])
            rope_back(sw, t1, ktsb[:, lg, :])

        # ---- phase 2: Q rounds (one head per round, paired halves) ----
        # Round r: proj(lh=r) 32 MMs (16 weight loads); rope_front(r);
        # rope_back(r-1); attn_head(r-1) fills PE while qp(r)'s usb copy
        # frees the pair for round r+1.
        for r in range(HPC):
            qp = ps_q.tile([P, 2, QBS], f32, tag="q", name="qp")
            usb = tmp.tile([P, T], bf16, tag="usb", name="usb")
            # interleave halves for kt<12 (shared weight loads); finish hf0
            # at kt 12..15 first so its evacuation overlaps the hf1 tail.
            for kt in range(12):
                for hf in range(2):
                    nc.tensor.matmul(
                        qp[:, hf, :],
                        wqsb[:, r, kt, :],
                        xtsb[:, kt, hf * QBS : (hf + 1) * QBS],
                        start=(kt == 0),
                        stop=False,
                    )
            for kt in range(12, KO):
                nc.tensor.matmul(
                    qp[:, 0, :],
                    wqsb[:, r, kt, :],
                    xtsb[:, kt, 0:QBS],
                    start=False,
                    stop=(kt == KO - 1),
                )
            nc.scalar.copy(usb[:, 0:QBS], qp[:, 0, :])
            for kt in range(12, KO):
                nc.tensor.matmul(
                    qp[:, 1, :],
                    wqsb[:, r, kt, :],
                    xtsb[:, kt, QBS:T],
                    start=False,
                    stop=(kt == KO - 1),
                )
            nc.vector.tensor_scalar_mul(usb[:, QBS:T], qp[:, 1, :], 1.0)
            if pend_rope:
                psw, pt1, pdst = pend_rope.pop()
                rope_back(psw, pt1, pdst)
            if r >= 1:
                attn_head(r - 1)
            sw, t1 = rope_swap(usb)
            pend_rope.append((sw, t1, qtsb[r][:, :]))
        sw, t1, pdst = pend_rope.pop()
        rope_back(sw, t1, pdst)

        # ---- phase 3: Wo, lh-outer with 4 column accumulators ----
        # Token tiles double-buffer: even tt uses the sp pool (2x2 banks),
        # odd tt uses ps_q + ps_ol (2+2 banks). One ot load -> 4 matmuls.
        # tt0's lh0..6 chains slot into attn_head(7)'s exp-ACT bubble
        # (they only need qb0 outputs of heads 0..6, long since final).
        attn_head(HPC - 1)
        dma_engines = [nc.sync, nc.gpsimd]
        for tt in range(TT):
            if tt % 2 == 0:
                ya = ps_sp.tile([P, 2, QBS], f32, tag="sp", name="ya")
                yb = ps_sp.tile([P, 2, QBS], f32, tag="sp", name="yb")
            else:
                ya = ps_q.tile([P, 2, QBS], f32, tag="q", name="ya")
                yb = ps_ol.tile([P, 2, QBS], f32, tag="ol", name="yb")
            yp = [ya[:, 0, :], ya[:, 1, :], yb[:, 0, :], yb[:, 1, :]]
            for lh in range(HPC):
                for cb in range(NCB):
                    nc.tensor.matmul(
                        yp[cb],
                        otsb[lh][:, tt * P : (tt + 1) * P],
                        wosb[:, cb, lh, :],
                        start=(lh == 0),
                        stop=(lh == HPC - 1),
                    )
            for cb in range(NCB):
                ysb = ysbp.tile([P, CBS], bf16, tag="ysb", name="ysb")
                if cb % 2 == 0:
                    nc.scalar.copy(ysb[:], yp[cb])
                else:
                    nc.vector.tensor_scalar_mul(ysb[:], yp[cb], 1.0)
                eng = dma_engines[(tt * NCB + cb) % 2]
                eng.dma_start(y_r[:, tt, cb * CBS : (cb + 1) * CBS], ysb[:])

    n_dedup = _dedup_ldweights(nc, mybir)
    nc.compile()
    return nc


def _get_program():
    if "p" not in _PROG_CACHE:
        _PROG_CACHE["p"] = _build_program()
    return _PROG_CACHE["p"]


def _prep_core(c, x, Wq, Wkv, Wo, cos, sin):
    import ml_dtypes

    mdt = ml_dtypes.bfloat16
    b = c // 2
    pair = c % 2
    groups = [2 * pair, 2 * pair + 1]
    heads = [g * G + i for g in groups for i in range(G)]

    xT = np.ascontiguousarray(x[b].T)                       # [DIM, T]
    xt_p = np.ascontiguousarray(xT.reshape(KO, P, T).transpose(1, 0, 2))

    wq_cols = np.stack([Wq[:, h * HD : (h + 1) * HD] for h in heads], axis=1)
    wq_p = np.ascontiguousarray(
        wq_cols.reshape(KO, P, HPC, HD).transpose(1, 2, 0, 3)
    )  # [P, lh, kt, c]

    kcols = np.concatenate([Wkv[:, g * HD : (g + 1) * HD] for g in groups], axis=1)
    wk_p = np.ascontiguousarray(kcols.reshape(KO, P, LG * HD).transpose(1, 0, 2))
    vcols = np.concatenate(
        [Wkv[:, KVH * HD + g * HD : KVH * HD + (g + 1) * HD] for g in groups], axis=1
    )
    wv_p = np.ascontiguousarray(vcols.reshape(KO, P, LG * HD).transpose(1, 0, 2))

    worows = np.stack([Wo[h * HD : (h + 1) * HD, :] for h in heads], axis=0)
    wo_p = np.ascontiguousarray(
        worows.reshape(HPC, P, NCB, CBS).transpose(1, 2, 0, 3)
    )  # [P, cb, lh, cc]

    cosT = np.ascontiguousarray(cos.T)                       # [64, T]
    sinT = np.ascontiguousarray(sin.T)
    cc_p = np.ascontiguousarray(np.concatenate([cosT, cosT], axis=0))   # [128, T]
    nss_p = np.ascontiguousarray(np.concatenate([-sinT, sinT], axis=0))
    tri_p = np.triu(np.ones((P, P), dtype=np.float32))
    trineg_p = -32768.0 * np.triu(np.ones((P, P), dtype=np.float32), k=1)
    idz_p = np.eye(P, dtype=np.float32)
    swp_p = np.roll(np.eye(P, dtype=np.float32), 64, axis=0)

    return {
        "xt": xt_p.astype(mdt),
        "wq": wq_p.astype(mdt),
        "wk": wk_p.astype(mdt),
        "wv": wv_p.astype(mdt),
        "wo": wo_p.astype(mdt),
        "cc": cc_p.astype(np.float32, copy=False),
        "nss": nss_p.astype(np.float32, copy=False),
        "tri": tri_p.astype(mdt),
        "trineg": trineg_p.astype(mdt),
        "idz": idz_p.astype(mdt),
        "swp": swp_p.astype(mdt),
    }


def _run(inputs, trace=False, trace_kwargs=None):
    from concourse import bass_utils

    x = np.asarray(inputs["x"], dtype=np.float32)
    Wq = np.asarray(inputs["Wq"], dtype=np.float32)
    Wkv = np.asarray(inputs["Wkv"], dtype=np.float32)
    Wo = np.asarray(inputs["Wo"], dtype=np.float32)
    cos = np.asarray(inputs["cos"], dtype=np.float32)
    sin = np.asarray(inputs["sin"], dtype=np.float32)

    nc = _get_program()
    in_maps = [_prep_core(c, x, Wq, Wkv, Wo, cos, sin) for c in range(NCORES)]
    kwargs = {}
    if trace:
        kwargs["trace"] = True
        if trace_kwargs:
            kwargs.update(trace_kwargs)
    res = bass_utils.run_bass_kernel_spmd(
        nc, in_maps, core_ids=list(range(NCORES)), **kwargs
    )
    outs = [np.asarray(r["y"]).astype(np.float32) for r in res.results]
    y = np.stack([outs[2 * b] + outs[2 * b + 1] for b in range(B)], axis=0)
    return y, res


def kernel(**inputs):
    y, _ = _run(inputs, trace=False)
    return y
